# revision 38
# baseline (speedup 1.0000x reference)
"""Trainium2 Bass kernel for nn_AuxiliaryModel_57707180589353.

Tree-conv GNN-ish model:
  - per-leaf 1x1 conv (scalar -> C channels) + leaf node weight
  - per-unmatched-column 1x1 conv
  - 10 levels of pairwise tree merge: Conv1d(C,C,3,'same') + BN(eval) + ReLU,
    scaled by per-node weight; every level emits a [B, C, 1024] feature chunk
  - concat all chunks along length, max-pool adjacent pairs, flatten.

Sharding: data-parallel over batch B=256 across 8 cores (32 samples/core).
All parameters are tiny and replicated.

Device layout (per core): activations live as [128, 1024] SBUF tiles:
  partition p = 16*s + c  (s = sample-in-group 0..7, c = channel 0..15),
  free dim   = spatial in "split" order: col j holds position 2j (even half,
  cols 0..511) / position 2j+1 (odd half, cols 512..1023).
The split order makes the conv taps contiguous matmuls and the final
pair-max-pool a dense tensor_tensor(max) of the two halves.

Conv1d(C,C,3) is computed on the TensorEngine as 3 accumulated matmuls with
block-diagonal (8 groups x 16x16) weights; per-node 'same' zero padding is
realized by subtracting the spurious cross-node-boundary contributions with
negated-weight matmuls over strided column APs.

Perf notes (v2):
  - x is cast to bf16 on the host; DMAs are ordered so the leaf-critical
    tensors land first and the big node-weight table streams in later.
  - a short burst of throwaway matmuls runs during the initial DMA wait so
    the PE HAM clock-gate is released (2.4 GHz) before real matmuls start.
  - leaf/unm stages use fused scalar_tensor_tensor on the vector engine;
    tree levels use ScalarE activation (relu+affine) + DVE node-weight mul;
    the pair-max pool is split between GpSimd and DVE to balance engines.
"""

import numpy as np
import ml_dtypes

B = 256
L = 1024
U = 256
C = 16
LEVELS = 10
EPS = 1e-5
N_CORES = 8
BPC = B // N_CORES          # 32 samples per core
SPG = 8                     # samples per matmul group (8*16 = 128 partitions)
GROUPS = BPC // SPG         # 4
T_OUT = (L + U + LEVELS * L) // 2   # 5760
OUT_COLS = C * T_OUT        # 92160

WARMUP_MMS = 12             # throwaway matmuls to release the PE clock gate

BF16 = ml_dtypes.bfloat16

_CACHE = {}


def _leaf_on_scalar(g):
    """Leaf groups whose PSUM->SBUF path runs on ScalarE (rest: DVE STT)."""
    return g >= 2


def _build_nc(reps=1):
    import concourse.bacc as bacc
    import concourse.tile as tile
    import concourse.mybir as mybir

    dt = mybir.dt
    f32 = dt.float32
    bf16 = dt.bfloat16
    Act = mybir.ActivationFunctionType
    Alu = mybir.AluOpType

    nc = bacc.Bacc("TRN2", target_bir_lowering=False, debug=False,
                   enable_asserts=False, num_devices=N_CORES)

    def din(name, shape, dtype=bf16):
        return nc.dram_tensor(name, list(shape), dtype, kind="ExternalInput").ap()

    x_d = din("x", [BPC, L + U])            # bf16 (host-cast)
    R_d = din("R", [32, GROUPS * 128])
    lwB_d = din("lwB", [128, L])
    lbB_d = din("lbB", [128, L])
    Wc_d = din("Wc", [128, 128])
    W0_d = din("W0", [128, 128])
    W2_d = din("W2", [128, 128])
    nW0_d = din("nW0", [128, 128])
    nW2_d = din("nW2", [128, 128])
    sP_d = din("sP", [128, 1], f32)
    b2P_d = din("b2P", [128, 1], f32)
    uwB_d = din("uwB", [128, U])
    ubB_d = din("ubB", [128, U])
    nwB_d = din("nwB", [128, LEVELS * L])
    out_d = nc.dram_tensor("out", [BPC, OUT_COLS], f32, kind="ExternalOutput").ap()

    # [2, 8, 16, 2, 5760] view of the output: (pair, sample, channel,
    # group-in-pair, pooled col) — row g*8+s = (2q+o)*8+s
    out_vp = out_d.rearrange("(q o s) (c t) -> q s c o t", o=2, s=SPG, c=C)

    with tile.TileContext(nc) as tc:
        with (tc.tile_pool(name="consts", bufs=1) as cpool,
              tc.tile_pool(name="work", bufs=2) as work,
              tc.tile_pool(name="curp", bufs=6) as curp,
              tc.tile_pool(name="poolp", bufs=12) as poolp,
              tc.tile_pool(name="psp", bufs=4, space="PSUM") as psp):
            # ---- PE warm-up: junk matmuls on a memset tile during DMA wait ----
            wsrc = cpool.tile([128, 512], bf16, name="wsrc")
            nc.vector.memset(wsrc, 0.0)
            wdst = psp.tile([128, L], f32, tag="ps", name="wdst")[:, 0:512]
            for i in range(WARMUP_MMS):
                nc.tensor.matmul(wdst, wsrc[:, 0:128], wsrc,
                                 start=True, stop=True, skip_group_check=True)

            # ---- input + constant loads (issue order = arrival order) ----
            x_sb = work.tile([BPC, L + U], bf16, tag="xstage", bufs=2,
                             name="xs")
            nc.sync.dma_start(out=x_sb, in_=x_d)
            R = cpool.tile_from(R_d)
            lwB = cpool.tile_from(lwB_d)
            lbB = cpool.tile_from(lbB_d)
            Wc = cpool.tile_from(Wc_d)
            W0 = cpool.tile_from(W0_d)
            W2 = cpool.tile_from(W2_d)
            sP = cpool.tile_from(sP_d)
            b2P = cpool.tile_from(b2P_d)
            # node weights: first two levels early, the rest streams later
            nwB = cpool.tile([128, LEVELS * L], bf16, name="nwB")
            nc.sync.dma_start(out=nwB[:, 0:2 * L], in_=nwB_d[:, 0:2 * L])
            nW0 = cpool.tile_from(nW0_d)
            nW2 = cpool.tile_from(nW2_d)
            uwB = cpool.tile_from(uwB_d)
            ubB = cpool.tile_from(ubB_d)
            nc.sync.dma_start(out=nwB[:, 2 * L:], in_=nwB_d[:, 2 * L:])

            def mm(out, lhsT, rhs, start, stop):
                nc.tensor.matmul(out, lhsT, rhs, start=start, stop=stop,
                                 skip_group_check=True)

            def pair_max(cur2, width, hwdge):
                """Pair-max the (even|odd) halves of both groups in cur2.
                hwdge chunks pool in f32 (HWDGE queue, no cast); the rest in
                bf16 (SWDGE queue, cast on the fly) — splitting the output
                traffic across two DMA queues."""
                v = cur2.rearrange("p (o h w) -> p o h w", o=2, h=2)
                pooled2 = poolp.tile([128, 2 * width], f32 if hwdge else bf16,
                                     tag="pooled", name="pl")
                nc.vector.tensor_tensor(
                    out=pooled2.rearrange("p (o w) -> p o w", o=2),
                    in0=v[:, :, 0, :], in1=v[:, :, 1, :], op=Alu.max)
                return pooled2

            def pooled_dma(pooled2, p, off, width, hwdge):
                eng = nc.sync if hwdge else nc.gpsimd
                eng.dma_start(out=out_vp[p, :, :, :, off:off + width],
                              in_=pooled2.rearrange("p (o w) -> p o w", o=2))

            for _rep in range(reps):
              xb = x_sb
              # ---- leaf stage: cur_{-1}[(s,c), j] = x[s,sig(j)]*lw'[sig(j),c] + lb' ----
              curs = [None] * GROUPS
              for p in range(2):
                  cur2 = curp.tile([128, 2 * L], bf16, tag="cur",
                                   name=f"curleaf{p}")
                  for i in range(2):
                      g = 2 * p + i
                      ps = psp.tile([128, L], f32, tag="ps", name=f"psleaf{g}")
                      Rg = R[:, g * 128:(g + 1) * 128]
                      mm(ps[:, 0:512], Rg, xb[:, 0:L:2], True, True)
                      mm(ps[:, 512:1024], Rg, xb[:, 1:L:2], True, True)
                      cur = cur2[:, i * L:(i + 1) * L]
                      if _leaf_on_scalar(g):
                          tmp = work.tile([128, L], bf16, tag="tmp", bufs=2,
                                          name=f"tmpleaf{g}")
                          nc.scalar.activation(out=tmp, in_=ps, func=Act.Copy,
                                               scale=1.0)
                          nc.vector.tensor_mul(out=cur, in0=tmp, in1=lwB)
                      else:
                          # cur = (ps * 1) * lwB    (fused on DVE)
                          nc.vector.scalar_tensor_tensor(
                              out=cur, in0=ps, scalar=1.0, in1=lwB,
                              op0=Alu.mult, op1=Alu.mult)
                      nc.vector.tensor_add(out=cur, in0=cur, in1=lbB)
                      curs[g] = cur
                  pooled2 = pair_max(cur2, 512, hwdge=False)
                  pooled_dma(pooled2, p, 0, 512, hwdge=False)

              # ---- tree levels ----
              def do_level(k, last=False):
                  hl = 1 << k          # half node length in split-layout columns
                  n = 512 >> k         # number of nodes at this level
                  nwk = nwB[:, k * L:(k + 1) * L]
                  for p in range(2):
                      if last:
                          # no next level: relu-affine then pool; act+mul on
                          # the 512 pooled cols only — halves the tail chain
                          pooled2 = poolp.tile([128, 2 * 512], bf16,
                                               tag="pooled", name=f"plL{p}")
                      else:
                          cur2 = curp.tile([128, 2 * L], bf16, tag="cur",
                                           name=f"cur{k}_{p}")
                      for i in range(2):
                          g = 2 * p + i
                          prev = curs[g]
                          ce, co = prev[:, 0:512], prev[:, 512:1024]
                          ps = psp.tile([128, L], f32, tag="ps",
                                        name=f"ps{k}_{g}")
                          pe, po = ps[:, 0:512], ps[:, 512:1024]
                          if k == 0:
                              # nodes are (leaf 2j, 2j+1); kernel-3 'same', len 2
                              mm(pe, Wc, ce, True, False)
                              mm(pe, W2, co, False, True)
                              mm(po, Wc, co, True, False)
                              mm(po, W0, ce, False, True)
                          elif k == 1:
                              # len-2 nodes in each half: W0/W2 taps are valid on
                              # alternate columns only -> strided MMs, no
                              # correction pass
                              mm(pe, Wc, ce, True, False)
                              mm(pe[:, 1:512:2], W0, co[:, 0:511:2], False, False)
                              mm(pe, W2, co, False, True)
                              mm(po, Wc, co, True, False)
                              mm(po[:, 0:511:2], W2, ce[:, 1:512:2], False, False)
                              mm(po, W0, ce, False, True)
                          elif k >= 4:
                              # long nodes: per-node segmented shift MMs skip the
                              # cross-node columns entirely (runs of hl-1 >= 15)
                              pev = pe.rearrange("p (b w) -> p b w", b=n)
                              pov = po.rearrange("p (b w) -> p b w", b=n)
                              cev = ce.rearrange("p (b w) -> p b w", b=n)
                              cov = co.rearrange("p (b w) -> p b w", b=n)
                              mm(pe, Wc, ce, True, False)
                              mm(pev[:, :, 1:hl], W0, cov[:, :, 0:hl - 1],
                                 False, False)
                              mm(pe, W2, co, False, True)
                              mm(po, Wc, co, True, False)
                              mm(pov[:, :, 0:hl - 1], W2, cev[:, :, 1:hl],
                                 False, False)
                              mm(po, W0, ce, False, True)
                          else:
                              # even outputs: pos 2j reads 2j-1 (odd j-1), 2j, 2j+1
                              mm(pe, Wc, ce, True, False)
                              mm(pe[:, 1:512], W0, co[:, 0:511], False, False)
                              # remove cross-node W0 term at node starts j*hl
                              mm(pe[:, hl:512:hl], nW0,
                                 co[:, hl - 1:511:hl], False, False)
                              mm(pe, W2, co, False, True)
                              # odd outputs: pos 2j+1 reads 2j, 2j+1, 2j+2
                              mm(po, Wc, co, True, False)
                              mm(po[:, 0:511], W2, ce[:, 1:512], False, False)
                              # remove cross-node W2 term at node ends j*hl-1
                              mm(po[:, hl - 1:511:hl], nW2,
                                 ce[:, hl:512:hl], False, False)
                              mm(po, W0, ce, False, True)
                          if last:
                              # relu-affine per half (pe act overlaps po MMs),
                              # then pool; nw-mul on the pooled 512 only
                              tmp = work.tile([128, L], bf16, tag="tmpl",
                                              bufs=4, name=f"tmpl{g}")
                              nc.scalar.activation(out=tmp[:, 0:512], in_=pe,
                                                   func=Act.Relu, bias=b2P,
                                                   scale=sP)
                              nc.scalar.activation(out=tmp[:, 512:1024],
                                                   in_=po, func=Act.Relu,
                                                   bias=b2P, scale=sP)
                              half = pooled2[:, i * 512:(i + 1) * 512]
                              nc.vector.tensor_tensor(
                                  out=half, in0=tmp[:, 0:512],
                                  in1=tmp[:, 512:1024], op=Alu.max)
                              nc.vector.tensor_mul(out=half, in0=half,
                                                   in1=nwk[:, 0:512])
                          else:
                              cur = cur2[:, i * L:(i + 1) * L]
                              nc.scalar.activation(out=cur, in_=ps,
                                                   func=Act.Relu,
                                                   bias=b2P, scale=sP)
                              nc.vector.tensor_mul(out=cur, in0=cur, in1=nwk)
                              curs[g] = cur
                      hw = (k % 2 == 0) and not last
                      if not last:
                          pooled2 = pair_max(cur2, 512, hwdge=hw)
                      pooled_dma(pooled2, p, 640 + 512 * k, 512, hwdge=hw)

              def do_unm():
                # ---- unmatched columns: pooled offset 512, w 128 ----
                for p in range(2):
                  tmpu2 = work.tile([128, 2 * U], bf16, tag="tmpu", bufs=2,
                                    name=f"tmpunm{p}")
                  for i in range(2):
                      g = 2 * p + i
                      psu = psp.tile([128, L], f32, tag="ps",
                                     name=f"psunm{g}")[:, 0:U]
                      Rg = R[:, g * 128:(g + 1) * 128]
                      mm(psu[:, 0:128], Rg, xb[:, L:L + U:2], True, True)
                      mm(psu[:, 128:256], Rg, xb[:, L + 1:L + U:2], True, True)
                      tmpu = tmpu2[:, i * U:(i + 1) * U]
                      nc.vector.scalar_tensor_tensor(
                          out=tmpu, in0=psu, scalar=1.0, in1=uwB,
                          op0=Alu.mult, op1=Alu.mult)
                      nc.vector.tensor_add(out=tmpu, in0=tmpu, in1=ubB)
                  pooledu2 = pair_max(tmpu2, 128, hwdge=True)
                  pooled_dma(pooledu2, p, 512, 128, hwdge=True)

              # unm slots two levels before the end so its vector/DMA tail
              # fully overlaps the final levels' matmuls
              for k in range(LEVELS - 2):
                  do_level(k)
              do_unm()
              do_level(LEVELS - 2)
              do_level(LEVELS - 1, last=True)

    nc.compile()
    return nc


def _split_cols(a):
    """Reorder the last axis from position order to split (even|odd) order."""
    return np.concatenate([a[..., 0::2], a[..., 1::2]], axis=-1)


def _host_consts(leaf_w, leaf_b, unm_w, unm_b, conv_w, conv_b,
                 bn_gamma, bn_beta, bn_mean, bn_var, leaf_nw, internal_nw):
    f32 = np.float32

    def rep_pc(v16):  # [16] -> [128, 1] (partition p = 16*s + c)
        return np.tile(np.asarray(v16, f32), SPG).reshape(128, 1)

    s = (bn_gamma / np.sqrt(bn_var + EPS)).astype(f32)
    b2 = ((conv_b - bn_mean) * s + bn_beta).astype(f32)

    lw = (leaf_w * leaf_nw[:, None]).astype(f32)      # [L, C]
    lb = (leaf_b * leaf_nw[:, None]).astype(f32)

    def bcast_cols(wLC):  # [Ncols, C] -> [128, Ncols] split order, bf16
        t = np.tile(wLC.T, (SPG, 1))                  # [128, Ncols]
        return _split_cols(t).astype(BF16)

    lwB = bcast_cols(lw)
    lbB = bcast_cols(lb)
    uwB = bcast_cols(np.asarray(unm_w, f32))
    ubB = bcast_cols(np.asarray(unm_b, f32))

    def blockdiag(w16):  # 16x16 block -> [128, 128] block-diagonal
        out = np.zeros((128, 128), f32)
        for g in range(SPG):
            out[g * C:(g + 1) * C, g * C:(g + 1) * C] = w16
        return out

    # lhsT[(g,ci),(g,co)] = conv_w[co, ci, k]
    Wk = [blockdiag(conv_w[:, :, k].T) for k in range(3)]
    Wc = Wk[1].astype(BF16)
    W0 = Wk[0].astype(BF16)
    W2 = Wk[2].astype(BF16)
    nW0 = (-Wk[0]).astype(BF16)
    nW2 = (-Wk[2]).astype(BF16)

    R = np.zeros((32, GROUPS * 128), f32)
    for g in range(GROUPS):
        for sl in range(SPG):
            R[g * SPG + sl, g * 128 + sl * C:g * 128 + (sl + 1) * C] = 1.0
    R = R.astype(BF16)

    # node-weight vectors per level, expanded to [128, 1024] in split order
    nwB = np.zeros((128, LEVELS * L), f32)
    off = 0
    for k in range(LEVELS):
        n = L >> (k + 1)
        w = np.asarray(internal_nw[off:off + n], f32)
        off += n
        expand = np.repeat(w, 1 << (k + 1))          # [1024] position order
        nwB[:, k * L:(k + 1) * L] = _split_cols(expand)[None, :]
    nwB = nwB.astype(BF16)

    return {
        "lwB": lwB, "lbB": lbB, "uwB": uwB, "ubB": ubB,
        "Wc": Wc, "W0": W0, "W2": W2, "nW0": nW0, "nW2": nW2,
        "R": R, "sP": rep_pc(s), "b2P": rep_pc(b2),
        "nwB": nwB,
    }


def kernel(x, leaf_w, leaf_b, unm_w, unm_b, conv_w, conv_b,
           bn_gamma, bn_beta, bn_mean, bn_var, leaf_nw, internal_nw):
    from concourse.bass_utils import run_bass_kernel_spmd

    if "nc" not in _CACHE:
        _CACHE["nc"] = _build_nc()
    nc = _CACHE["nc"]

    consts = _host_consts(
        np.asarray(leaf_w), np.asarray(leaf_b), np.asarray(unm_w),
        np.asarray(unm_b), np.asarray(conv_w), np.asarray(conv_b),
        np.asarray(bn_gamma), np.asarray(bn_beta), np.asarray(bn_mean),
        np.asarray(bn_var), np.asarray(leaf_nw), np.asarray(internal_nw))

    x = np.ascontiguousarray(np.asarray(x, np.float32).astype(BF16))
    in_maps = []
    for c in range(N_CORES):
        m = dict(consts)
        m["x"] = np.ascontiguousarray(x[c * BPC:(c + 1) * BPC])
        in_maps.append(m)

    res = run_bass_kernel_spmd(nc, in_maps, core_ids=list(range(N_CORES)))
    out = np.concatenate([r["out"] for r in res.results], axis=0)
    return out.astype(np.float32)
